# revision 2
# baseline (speedup 1.0000x reference)
"""ChannelShift kernel for Trainium2 (Bass), data-parallel over 8 NeuronCores.

Reference op (per sample, x viewed as [C, H*W] row-major):
  cols [0, FOLD)       : out[t] = x[t+1]  (zero at t=C-1)   -- shift left
  cols [FOLD, 2*FOLD)  : out[t] = x[t-1]  (zero at t=0)     -- shift right
  cols [2*FOLD, HW)    : out[t] = x[t]                       -- identity

Pure data movement: implemented as 3 strided DRAM->DRAM DMA copies plus 2
tiny zero-fill DMAs (from a const-zero tensor baked into the NEFF). All 5
transfers write disjoint output regions, so they run fully in parallel on
the HWDGE ring with a single final semaphore wait.

Sharding: batch 64 -> 8 samples per core, no cross-core communication.
"""

import numpy as np

import concourse.bass as bass
import concourse.mybir as mybir
from concourse.bass_utils import run_bass_kernel_spmd

BS, C, H, W = 64, 512, 56, 56
HW = H * W              # 3136
FOLD = HW // 8          # 392
N_CORES = 8
BS_PER = BS // N_CORES  # 8

_nc_cache = None


def _build_nc() -> bass.Bass:
    nc = bass.Bass()
    x = nc.declare_dram_parameter("x", [BS_PER, C, HW], mybir.dt.float32, isOutput=False)
    out = nc.declare_dram_parameter("out", [BS_PER, C, HW], mybir.dt.float32, isOutput=True)
    zeros = nc.inline_tensor(np.zeros((BS_PER, FOLD), np.float32), name="zeros")

    with nc.Block() as block, nc.semaphore("dma_sem") as dma_sem:

        @block.sync
        def _(sync):
            # A DMA's descriptor count is bs*rows; the HWDGE only spreads a
            # DMA across all 16 SDMA engines when that count divides by 16
            # (8*511 -> only 8 engines; 8*510 and 8*512 -> 16). So each
            # 511-row shift band is a 510-row main copy + a 1-row tail.
            n = 0

            def dma(o, i):
                nonlocal n
                sync.dma_start(out=o, in_=i).then_inc(dma_sem, 16)
                n += 16

            # shift left: out[:, t, 0:FOLD] = x[:, t+1, 0:FOLD], t in [0, C-1)
            dma(out[:, 0 : C - 2, 0:FOLD], x[:, 1 : C - 1, 0:FOLD])
            dma(out[:, C - 2, 0:FOLD], x[:, C - 1, 0:FOLD])
            dma(out[:, C - 1, 0:FOLD], zeros[:, :])
            # shift right: out[:, t, FOLD:2F] = x[:, t-1, FOLD:2F], t in [1, C)
            dma(out[:, 1 : C - 1, FOLD : 2 * FOLD], x[:, 0 : C - 2, FOLD : 2 * FOLD])
            dma(out[:, C - 1, FOLD : 2 * FOLD], x[:, C - 2, FOLD : 2 * FOLD])
            dma(out[:, 0, FOLD : 2 * FOLD], zeros[:, :])
            # identity tail: out[:, :, 2*FOLD:] = x[:, :, 2*FOLD:]
            dma(out[:, :, 2 * FOLD :], x[:, :, 2 * FOLD :])
            sync.wait_ge(dma_sem, n)

    return nc


def _run(x: np.ndarray, trace: bool = False):
    """Shard, execute on 8 cores, return (full_output, BassKernelResults)."""
    global _nc_cache
    if _nc_cache is None:
        _nc_cache = _build_nc()
    nc = _nc_cache

    x3 = np.ascontiguousarray(np.asarray(x, dtype=np.float32).reshape(BS, C, HW))
    in_maps = [
        {"x": x3[i * BS_PER : (i + 1) * BS_PER]} for i in range(N_CORES)
    ]
    res = run_bass_kernel_spmd(nc, in_maps, list(range(N_CORES)), trace=trace)
    out = np.concatenate([r["out"] for r in res.results], axis=0)
    return out.reshape(BS, C, H, W), res


def kernel(x: np.ndarray) -> np.ndarray:
    out, _ = _run(x, trace=False)
    return out
